# revision 45
# baseline (speedup 1.0000x reference)
"""Single-head causal attention (B=4, T=4096, E=1024, H=64) on 8 TRN2 NeuronCores.

One SPMD module, one launch, identical code on all 8 cores; per-core behavior
is steered entirely by input data. Core c handles batch c//2 with key-parity
j = c%2: it owns the odd/even 128-wide key tiles and computes, for EVERY
512-query block of its batch, partial attention (numerator + denominator) over
its own key tiles only. The two cores of a batch are exactly complementary;
the host adds the partials, normalizes, and un-permutes columns.

Per-block own s-tiles form (b+1) hi/lo PAIRS: kT is staged with the pair's
even tile in SBUF partitions 0:63 and the odd tile in 64:127, and qT is
duplicated into both partition halves (via a [Wq|Wq] stationary matrix), so
the two K=64 score matmuls of a pair run CONCURRENTLY in the two PE row-group
halves (measured 139 ns/matmul vs 434 serialized). All matmul inputs are
fp16/bf16 - fp32 modes run 1.5-3x slower and poison FWL for neighbors.

exp() runs on ACT over [128,1024] pair tiles (PSUM pair = 2 adjacent banks)
to amortize the fixed activation overhead; causal masking multiplies the
diagonal pair by a precomputed bf16 0/1 mask (input data, per-core). Block
columns are ordered [own1|other1|own2|other2] so the diagonal pair's hi tile
only reaches cols 256:512 on both parities: its score matmul, exp and PV run
on 256/768 columns, and only the two masked 256-col ends are multiplied. PV
accumulates [65,512] per block in PSUM (V' carries a ones column so the
softmax denominator rides along) and lags one pair behind the score/exp
stream GLOBALLY (across blocks) so diagonal chains hide under the next block.

Scheduling: the HAM clock gate holds the PE at 1.2 GHz until ~3.4us of
sustained matmul activity, so dummy warm-up matmuls run under the initial x
DMAs (and filler matmuls bridge the sc0-h1 wait). The 16 DMA engines are
shared across queues (~270 GB/s aggregate), so x stays on the sync queue in
arrival=consumption order while the (tiny) weights ride scalar. Projection
feeds are deadline-scheduled: q_k gates block 2k's scores and kv_k gates pair
(2k,2k), so each group releases between pairs as late as its deadline allows,
keeping filler PE work available in the ACT-bound late blocks. Output is
bf16 (host normalizes in fp32).
"""

import math
import numpy as np
import ml_dtypes

import concourse.bacc as bacc
import concourse.tile as tile
import concourse.mybir as mybir
from concourse.bass_utils import run_bass_kernel_spmd
from concourse.masks import make_identity

f32 = mybir.dt.float32
bf16 = mybir.dt.bfloat16
fp16 = mybir.dt.float16

B, T, E, H = 4, 4096, 1024, 64
NCORES = 8
NSC = 4            # super-chunks of 1024 query/key columns
NE = E // 128      # contraction tiles
NB = T // 512      # 512-query blocks
NPAIR = NB         # own-key 128-tile pairs (pair p is block p's diagonal)


def _tile_order(sc, j):
    """Global 128-col tile indices for the 8 slots of super-chunk sc, parity j."""
    o = 8 * sc
    return [o + j, o + 4 + j, o + 2 + j, o + 6 + j,
            o + 1 - j, o + 5 - j, o + 3 - j, o + 7 - j]


def _permq(j):
    """Block-relative q offset for each of the 512 assembled block columns.

    Column order [own1 | other1 | own2 | other2] puts every column the
    diagonal pair's HI tile can reach in cols 256:512 for BOTH parities, so
    the diagonal hi score matmul / exp / PV run on 256 columns only.
    """
    c = np.arange(128)
    out = np.empty(512, np.int64)
    out[0:128] = 128 * j + c
    out[128:256] = 128 * (1 - j) + c
    out[256:384] = 256 + 128 * j + c
    out[384:512] = 256 + 128 * (1 - j) + c
    return out


def build_nc(level=3):
    nc = bacc.Bacc(name=f"attn8_l{level}")
    x_d = nc.dram_tensor("x", [NSC, 2, 128, NE, 512], fp16, kind="ExternalInput")
    wa_d = nc.dram_tensor("wa", [128, NE, 128], fp16, kind="ExternalInput")
    wq_d = nc.dram_tensor("wq", [128, NE, 128], fp16, kind="ExternalInput")
    mask_d = nc.dram_tensor("mask", [128, 512], bf16, kind="ExternalInput")
    out_d = nc.dram_tensor("out", [H + 1, NB, 512], bf16, kind="ExternalOutput")

    with tile.TileContext(nc) as tc:
        with tc.tile_pool(name="singles", bufs=1) as singles, \
             tc.tile_pool(name="work", bufs=2) as work, \
             tc.tile_pool(name="pwork", bufs=4) as pwork, \
             tc.tile_pool(name="psS", bufs=2, space="PSUM") as psS, \
             tc.tile_pool(name="psO", bufs=2, space="PSUM") as psO, \
             tc.tile_pool(name="psP", bufs=2, space="PSUM") as psP:

            # weights/mask on the scalar DGE queue so their issue overlaps the
            # x DMAs on sync; per (super, half) x pieces are host-packed
            # contiguous (one 8KB descriptor per partition), own half first.
            wa_sb = singles.tile([128, NE, 128], fp16)
            nc.scalar.dma_start(out=wa_sb[:, 0, :], in_=wa_d[:, 0, :])
            nc.scalar.dma_start(out=wa_sb[:, 1:NE, :], in_=wa_d[:, 1:NE, :])
            wq_sb = singles.tile([128, NE, 128], fp16)
            nc.scalar.dma_start(out=wq_sb, in_=wq_d[:, :, :])
            mask_sb = singles.tile([128, 512], bf16)
            ident = singles.tile([128, 128], bf16)
            make_identity(nc, ident)

            # PE warm-up: the HAM clock gate holds the PE at 1.2 GHz until it
            # sees ~3.4us of sustained matmul activity, which otherwise only
            # happens ~13us in (after the first x super-chunk lands). Burn
            # dummy matmuls into a scratch PSUM tile while the DMAs are in
            # flight so the real projections run at 2.4 GHz. Also fire one
            # dummy exp so the ACT spline-table DMA happens now, not in front
            # of the first real softmax.
            warm_sb = singles.tile([128, 512], bf16)
            nc.vector.memset(warm_sb, 0.0)
            dume = singles.tile([128, 1], f32)
            nc.scalar.activation(dume, warm_sb[:, 0:1],
                                 mybir.ActivationFunctionType.Exp)
            warm_ps = psS.tile([128, 1024], f32, tag="s", name="warm")
            for _ in range(12):
                nc.tensor.matmul(warm_ps[:, 0:512], ident, warm_sb,
                                 start=True, stop=True)

            # All x on the sync queue in consumption order: the 16 DMA
            # engines are shared across queues (~270 GB/s aggregate), so a
            # second queue adds no bandwidth and only lets x packets crowd
            # out the weights the first projections wait on. sc0 goes in
            # fine pieces (the first projections chase them); later halves
            # are single 1MB descriptors. gpsimd DMA is the software DGE at
            # ~4x lower throughput — never route bulk x through it.
            x_sb = singles.tile([128, NSC, 2, NE, 512], fp16)
            nc.scalar.dma_start(out=mask_sb, in_=mask_d[:, :])
            for e0 in range(0, 8, 4):
                nc.sync.dma_start(out=x_sb[:, 0, 0, e0:e0 + 4],
                                  in_=x_d[0, 0, :, e0:e0 + 4])
            for e0 in range(0, 8, 4):
                nc.sync.dma_start(out=x_sb[:, 0, 1, e0:e0 + 4],
                                  in_=x_d[0, 1, :, e0:e0 + 4])
            # Piece sizes are a latency/bandwidth tradeoff: each extra
            # DMA_DIRECT2D costs ~0.65us of descriptor time that does NOT
            # overlap the transfers, so fine-grained pieces throttle the
            # stream (measured: 2-etile pieces push every block start ~3us
            # later). 0.5-1MB pieces keep the stream at full bandwidth.
            for h in range(2):
                nc.sync.dma_start(out=x_sb[:, 1, h], in_=x_d[1, h])
            for sc in range(2, NSC):
                for h in range(2):
                    for e0 in range(0, 8, 4):
                        nc.sync.dma_start(out=x_sb[:, sc, h, e0:e0 + 4],
                                          in_=x_d[sc, h, :, e0:e0 + 4])

            kT_sb = singles.tile([128, NPAIR, 128], fp16)
            qT_sb = singles.tile([128, NB, 512], fp16)
            vp_sb = singles.tile([128, 2 * NPAIR, H + 1], bf16)
            nc.vector.memset(vp_sb[:, :, H:H + 1], 1.0)
            oT_sb = singles.tile([H + 1, NB, 512], bf16)

            # ---- projection emission units (interleavable closures) ----
            def kv_units(sc):
                kv = [None]

                def mm(e):
                    def f():
                        if e == 0:
                            kv[0] = psP.tile([128, 512], f32, tag="pr",
                                             name=f"kv{sc}", uniquify=True)
                        nc.tensor.matmul(kv[0], wa_sb[:, e, :],
                                         x_sb[:, sc, 0, e, :],
                                         start=(e == 0), stop=(e == NE - 1))
                    return f

                def evac():
                    nc.vector.tensor_copy(kT_sb[0:64, 2 * sc:2 * sc + 2, :],
                                          kv[0][0:64, 0:256])
                    nc.vector.tensor_copy(kT_sb[64:128, 2 * sc:2 * sc + 2, :],
                                          kv[0][0:64, 256:512])
                    vT = work.tile([128, 512], bf16, tag="vt", name=f"vT{sc}")
                    nc.vector.tensor_copy(vT[64:128, :], kv[0][64:128, :])
                    kv.append(vT)

                def transp(k, i_off):
                    def f():
                        vT = kv[-1]
                        vt = psP.tile([128, 64], bf16, tag="pr",
                                      name=f"vt{sc}_{k}")
                        nc.tensor.transpose(vt, vT[64:128, 128 * k:128 * (k + 1)],
                                            ident[64:128, 64:128])
                        nc.vector.tensor_copy(vp_sb[:, 4 * sc + i_off, 0:H], vt)
                    return f

                return [mm(e) for e in range(NE)] + [evac] + \
                    [transp(k, i_off) for k, i_off in
                     ((0, 0), (2, 1), (1, 2), (3, 3))]

            def q_units(sc):
                ps = {}

                def mm(half, e):
                    def f():
                        if e == 0:
                            ps[half] = psP.tile([128, 512], f32, tag="pr",
                                                name=f"q{half}{sc}")
                        nc.tensor.matmul(ps[half], wq_sb[:, e, :],
                                         x_sb[:, sc, half, e, :],
                                         start=(e == 0), stop=(e == NE - 1))
                    return f

                def qcopy():
                    # block cols [own1|other1|own2|other2]: halves A (own
                    # parity) and B (other) interleave at 128 granularity.
                    qA, qB = ps[0], ps[1]
                    for hb in range(2):
                        b = 2 * sc + hb
                        nc.vector.tensor_copy(qT_sb[:, b, 0:128], qA[:, hb * 128:(hb + 1) * 128])
                        nc.vector.tensor_copy(qT_sb[:, b, 128:256], qB[:, hb * 128:(hb + 1) * 128])
                        nc.vector.tensor_copy(qT_sb[:, b, 256:384], qA[:, 256 + hb * 128:256 + (hb + 1) * 128])
                        nc.vector.tensor_copy(qT_sb[:, b, 384:512], qB[:, 256 + hb * 128:256 + (hb + 1) * 128])

                return [mm(h, e) for h in range(2) for e in range(NE)] + [qcopy]

            # ---- attention with a proj-unit feed interleaved per pair ----
            # PV lags one pair behind the score/exp stream GLOBALLY (across
            # blocks and stages), so the diagonal pair's exp->mask->PV chain
            # hides under the next block's first score matmuls instead of
            # stalling the PE once per block.
            lag = [None]

            def emit_pv(p, pp, o_ps, b, final=False):
                if p == b and final:
                    # last pair of the kernel: split PV-lo so the unmasked
                    # half overlaps the mask multiply on the critical tail
                    nc.tensor.matmul(o_ps[:, 256:512], vp_sb[:, 2 * p, :],
                                     pp[:, 256:512], start=(p == 0), stop=False)
                    nc.tensor.matmul(o_ps[:, 0:256], vp_sb[:, 2 * p, :],
                                     pp[:, 0:256], start=False, stop=False)
                else:
                    nc.tensor.matmul(o_ps, vp_sb[:, 2 * p, :], pp[:, 0:512],
                                     start=(p == 0), stop=False)
                if p == b:
                    # diagonal hi tile reaches only block cols 256:512
                    nc.tensor.matmul(o_ps[:, 256:512], vp_sb[:, 2 * p + 1, :],
                                     pp[:, 512:768], start=False, stop=True)
                    nc.vector.tensor_copy(oT_sb[:, b, :], o_ps)
                    nc.sync.dma_start(out=out_d[:, b, :], in_=oT_sb[:, b, :])
                else:
                    nc.tensor.matmul(o_ps, vp_sb[:, 2 * p + 1, :],
                                     pp[:, 512:1024], start=False, stop=False)

            def drain_lag_final():
                if lag[0] is not None:
                    emit_pv(*lag[0], final=True)
                    lag[0] = None

            def drain_lag():
                if lag[0] is not None:
                    emit_pv(*lag[0])
                    lag[0] = None

            def attention_block(b, feed):
                o_ps = psO.tile([H + 1, 512], f32, tag="o", name=f"o{b}")

                for p in range(b + 1):
                    s_ps = psS.tile([128, 1024], f32, tag="s", name=f"s{b}_{p}")
                    nc.tensor.matmul(s_ps[:, 0:512], kT_sb[0:64, p, :],
                                     qT_sb[0:64, b, :], start=True, stop=True)
                    pp = pwork.tile([128, 1024], bf16, tag="p", name=f"p{b}_{p}")
                    if p == b:
                        nc.tensor.matmul(s_ps[:, 512:768], kT_sb[64:128, p, :],
                                         qT_sb[64:128, b, 256:512],
                                         start=True, stop=True)
                        nc.scalar.activation(pp[:, 0:768], s_ps[:, 0:768],
                                             mybir.ActivationFunctionType.Exp)
                        # block cols 256:512 are causal-complete for the lo
                        # tile on both parities — only mask the two ends
                        nc.vector.tensor_mul(pp[:, 0:256], pp[:, 0:256],
                                             mask_sb[:, 0:256])
                        nc.vector.tensor_mul(pp[:, 512:768], pp[:, 512:768],
                                             mask_sb[:, 256:512])
                    else:
                        nc.tensor.matmul(s_ps[:, 512:1024], kT_sb[64:128, p, :],
                                         qT_sb[64:128, b, :], start=True, stop=True)
                        nc.scalar.activation(pp, s_ps,
                                             mybir.ActivationFunctionType.Exp)
                    for u in feed(p, b):
                        u()
                    drain_lag()
                    lag[0] = (p, pp, o_ps, b)

            def run_stage(blocks, groups):
                """Emit the blocks' attention; `groups` is a list of
                (units, finish_by): each group's units are released across
                pairs (prev_finish, finish_by] — finish_by is the pair count
                by which the group must be fully emitted (its true deadline),
                so late ACT-bound pairs still get PE filler work."""
                done_pairs = 0
                state = []
                g0 = 0
                for units, fby in groups:
                    state.append([list(units), len(units), g0, fby])
                    g0 = fby

                def feed(p, b):
                    nonlocal done_pairs
                    done_pairs += 1
                    out = []
                    for st in state:
                        queue, n, a, z = st
                        if not queue:
                            continue
                        if done_pairs >= z:
                            out.extend(queue)
                            queue.clear()
                            continue
                        if done_pairs <= a:
                            break
                        want = (n * (done_pairs - a) + (z - a) - 1) // (z - a)
                        emitted = n - len(queue)
                        # release in bursts of >=3 so consecutive projection
                        # matmuls keep their weight-load streams pipelined
                        if want - emitted < 3:
                            break
                        while emitted < want and queue:
                            out.append(queue.pop(0))
                            emitted += 1
                        break
                    return out

                for b in blocks:
                    attention_block(b, feed)
                for st in state:
                    for u in st[0]:
                        u()

            # stage -1: project super 0 outright (nothing to overlap with).
            # Interleave the kv and q-half0 matmuls: both consume the same
            # x pieces in e-order, so the PE chases the DMA stream without
            # per-unit stalls long enough to re-trip the HAM throttle.
            kv0, q0 = kv_units(0), q_units(0)
            for e in range(NE):
                kv0[e]()
                q0[e]()
            kv0[NE]()                    # kv evac (DVE) overlaps q half1
            # filler matmuls bridge the wait for the x h1 pieces so the PE
            # stays busy and the HAM doesn't re-throttle to half clock
            for _ in range(6):
                nc.tensor.matmul(warm_ps[:, 0:512], ident, warm_sb,
                                 start=True, stop=True)
            for u in q0[NE:2 * NE]:      # q half1 matmuls
                u()
            q0[2 * NE]()                 # qcopy first: block 0 scores wait it
            for u in kv0[NE + 1:]:       # v transposes
                u()
            # stage sc: attention of super sc's blocks, interleaved with the
            # next super's projections. Super 3's K/V is deferred into the
            # final stage so the tail exp burst still has PE work around it.
            # Feeds are deadline-scheduled: q_k gates block 2k's scores, kv_k
            # gates pair (2k, 2k) — so each group rides as late as its
            # deadline allows, letting pairs stream at ACT pace and keeping
            # filler PE work available in the ACT-bound late blocks. The
            # q-h1 convoys sit between stages (block 2k can't start earlier
            # anyway); ACT drains its backlog under them.
            kv1, kv2, kv3 = kv_units(1), kv_units(2), kv_units(3)
            q1, q2, q3 = q_units(1), q_units(2), q_units(3)
            run_stage([0, 1], [(q1[0:NE], 2), (q1[NE:], 3)])
            run_stage([2, 3], [(kv1, 2), (q2[0:NE], 5), (q2[NE:], 7)])
            run_stage([4, 5], [(kv2, 4), (q3[0:NE], 9), (q3[NE:], 11)])
            # the only ACT-bound stage: flatten kv3 so early pairs aren't
            # slowed below exp pace. kT evac gates scores(6,6) (pair 7);
            # pair-6 vp transposes gate PV(6,6), drained at pair 8's feed;
            # pair-7 vp transposes gate only the final PV(7,7) drain.
            run_stage([6, 7], [(kv3[0:9], 6), (kv3[9:11], 8),
                               (kv3[11:13], 15)])
            drain_lag_final()

    nc.finalize()
    return nc


_NC_CACHE = {}


def _prep_core(x_b, j):
    """Per-core permuted xT layout [NSC, 2, 128, NE, 512] fp16."""
    xT = np.ascontiguousarray(x_b.T).astype(np.float16)  # [E, T]
    out = np.empty((NSC, 2, 128, NE, 512), np.float16)
    for sc in range(NSC):
        cols = np.concatenate([np.arange(128) + 128 * g for g in _tile_order(sc, j)])
        sel = xT[:, cols]                                # [E, 1024]
        s4 = sel.reshape(NE, 128, 2, 512).transpose(2, 1, 0, 3)  # [2,128,NE,512]
        out[sc] = s4
    return out


def _make_masks(j):
    permq = _permq(j)
    r = np.arange(128)[:, None]
    m_lo = (128 * j + r) <= permq[None, 0:256]      # cols 256:512 are all-1
    m_hi = (256 + 128 * j + r) <= permq[None, 256:512]
    return np.concatenate([m_lo, m_hi], axis=1).astype(ml_dtypes.bfloat16)


def run(x, Wq, Wk, Wv, trace=False):
    x = np.asarray(x, np.float32)
    Wk_, Wv_, Wq_ = (np.asarray(w, np.float32) for w in (Wk, Wv, Wq))
    wa = np.concatenate([Wk_, Wv_], 1).astype(np.float16).reshape(NE, 128, 128).transpose(1, 0, 2)
    wq = np.concatenate([Wq_, Wq_], 1).astype(np.float16).reshape(NE, 128, 128).transpose(1, 0, 2)
    wa, wq = (np.ascontiguousarray(w) for w in (wa, wq))
    masks = [_make_masks(j) for j in range(2)]

    if "nc" not in _NC_CACHE:
        _NC_CACHE["nc"] = build_nc()
    nc = _NC_CACHE["nc"]

    in_maps = []
    for c in range(NCORES):
        bb, j = c // 2, c % 2
        in_maps.append({"x": _prep_core(x[bb], j), "wa": wa,
                        "wq": wq, "mask": masks[j]})
    res = run_bass_kernel_spmd(nc, in_maps, core_ids=list(range(NCORES)),
                               trace=trace)

    permqs = [_permq(0), _permq(1)]
    inv_sqrt_h = 1.0 / math.sqrt(H)
    full = np.empty((B, T, H), np.float32)
    for bb in range(B):
        num = np.zeros((H + 1, T), np.float32)
        for j in range(2):
            o = np.asarray(res.results[2 * bb + j]["out"],
                           np.float32)                   # [H+1, NB, 512]
            cols = (np.arange(NB)[:, None] * 512 + permqs[j][None, :]).ravel()
            num[:, cols] += o.reshape(H + 1, NB * 512)
        full[bb] = (num[0:H] / num[H:H + 1] * inv_sqrt_h).T
    return full, res


def kernel(x, Wq, Wk, Wv):
    out, _ = run(x, Wq, Wk, Wv)
    return out

